# revision 5
# baseline (speedup 1.0000x reference)
"""MoE FFN (FMoE) kernel for 8 Trainium2 NeuronCores.

Problem: N=4096 tokens, D=512, H=2048, E=8 experts, top_k=2.
  logits = inp @ gate_w + gate_b ; top-2 softmax -> combine weights
  out = sum_e combine[:, e] * (gelu_tanh(inp @ w1[e] + b1[e]) @ w2[e] + b2[e])

Strategy (dense data-parallel): each core owns N/8 = 512 tokens and runs
the full gate + all-8-expert FFN on its slice; no cross-core traffic.
Main matmuls run as float32r (fast fp32 mode, ~1e-4 rel err); the gate
matmul runs exact fp32 so top-2 selection matches the reference.
"""
import numpy as np

import concourse.bacc as bacc
import concourse.bass as bass
import concourse.mybir as mybir
import concourse.tile as tile
from concourse.bass_utils import run_bass_kernel_spmd

N, D, H, E, TOPK = 4096, 512, 2048, 8, 2
M = 8              # cores
TN = N // M        # tokens per core
P = 128
DC = D // P        # 4 contraction chunks over D
HC = H // P        # 16 chunks over H
TC = TN // P       # 4 token chunks per core

FP32 = mybir.dt.float32
FP32R = mybir.dt.float32r
U32 = mybir.dt.uint32

AFT = mybir.ActivationFunctionType


def _gate_combine(nc, tc_ctx, pools, xts, gws, gb, ones_s, iota_u, n_tok_chunks):
    """Emit gate matmul + top-2 softmax; returns list of combine tiles [P, E]."""
    gatep, cmbp, psg = pools
    cmb = []
    for t in range(n_tok_chunks):
        pg = psg.tile([P, E], FP32)
        for dc in range(len(xts)):
            nc.tensor.matmul(pg[:], xts[dc][:, t * P:(t + 1) * P], gws[dc][:],
                             start=(dc == 0), stop=False)
        nc.tensor.matmul(pg[:], ones_s[:], gb[:], start=False, stop=True)

        lg = gatep.tile([P, E], FP32, tag="lg")
        nc.vector.tensor_copy(lg[:], pg[:])
        mx = gatep.tile([P, 8], FP32, tag="mx")
        ix = gatep.tile([P, 8], U32, tag="ix")
        nc.vector.max_with_indices(mx[:], ix[:], lg[:])

        dlt = gatep.tile([P, 1], FP32, tag="dlt")
        nc.vector.tensor_sub(dlt[:], mx[:, 1:2], mx[:, 0:1])
        e1 = gatep.tile([P, 1], FP32, tag="e1")
        nc.scalar.activation(e1[:], dlt[:], AFT.Exp)
        den = gatep.tile([P, 1], FP32, tag="den")
        nc.vector.tensor_scalar_add(den[:], e1[:], 1.0)
        w0 = gatep.tile([P, 1], FP32, tag="w0")
        nc.vector.reciprocal(w0[:], den[:])
        w1_ = gatep.tile([P, 1], FP32, tag="w1_")
        nc.vector.tensor_mul(w1_[:], e1[:], w0[:])

        oh0 = gatep.tile([P, E], FP32, tag="oh0")
        nc.vector.tensor_tensor(out=oh0[:], in0=ix[:, 0:1].to_broadcast([P, E]),
                                in1=iota_u[:], op=mybir.AluOpType.is_equal)
        oh1 = gatep.tile([P, E], FP32, tag="oh1")
        nc.vector.tensor_tensor(out=oh1[:], in0=ix[:, 1:2].to_broadcast([P, E]),
                                in1=iota_u[:], op=mybir.AluOpType.is_equal)
        nc.vector.tensor_scalar_mul(oh0[:], oh0[:], w0[:, 0:1])
        nc.vector.tensor_scalar_mul(oh1[:], oh1[:], w1_[:, 0:1])
        c = cmbp.tile([P, E], FP32, tag="cmb")
        nc.vector.tensor_add(c[:], oh0[:], oh1[:])
        cmb.append(c)
    return cmb


def build_dense():
    nc = bacc.Bacc(None, target_bir_lowering=False)

    xT_r = nc.dram_tensor("xT_r", [D, TN], FP32R, kind="ExternalInput")
    xT_s = nc.dram_tensor("xT_s", [D, TN], FP32, kind="ExternalInput")
    gate_w = nc.dram_tensor("gate_w", [D, E], FP32, kind="ExternalInput")
    gate_b = nc.dram_tensor("gate_b", [1, E], FP32, kind="ExternalInput")
    w1 = nc.dram_tensor("w1", [E, D, H], FP32R, kind="ExternalInput")
    b1p = nc.dram_tensor("b1p", [E, P, HC], FP32, kind="ExternalInput")
    w2 = nc.dram_tensor("w2", [E, H, D], FP32R, kind="ExternalInput")
    b2 = nc.dram_tensor("b2", [E, 1, D], FP32R, kind="ExternalInput")
    ones_in = nc.dram_tensor("ones_in", [1, P], FP32R, kind="ExternalInput")
    out = nc.dram_tensor("out", [TN, D], FP32, kind="ExternalOutput")

    with tile.TileContext(nc) as tc:
        with (
            tc.tile_pool(name="xpool", bufs=DC) as xpool,
            tc.tile_pool(name="const", bufs=1) as const,
            tc.tile_pool(name="gatep", bufs=2) as gatep,
            tc.tile_pool(name="cmbp", bufs=TC) as cmbp,
            tc.tile_pool(name="w1p", bufs=3) as w1p,
            tc.tile_pool(name="w2p", bufs=2 * HC) as w2p,
            tc.tile_pool(name="hp", bufs=2 * HC) as hp,
            tc.tile_pool(name="accp", bufs=TC) as accp,
            tc.tile_pool(name="tmpp", bufs=3) as tmpp,
            tc.tile_pool(name="bp", bufs=4) as bp,
            tc.tile_pool(name="psg", bufs=2, space="PSUM") as psg,
            tc.tile_pool(name="ps1", bufs=2, space="PSUM") as ps1,
            tc.tile_pool(name="ps2", bufs=2, space="PSUM") as ps2,
        ):
            # ---- resident inputs ----
            xtr, xts = [], []
            for dc in range(DC):
                tr = xpool.tile([P, TN], FP32R, tag="xtr")
                nc.sync.dma_start(tr[:], xT_r[dc * P:(dc + 1) * P, :])
                xtr.append(tr)
                ts = xpool.tile([P, TN], FP32, tag="xts")
                nc.sync.dma_start(ts[:], xT_s[dc * P:(dc + 1) * P, :])
                xts.append(ts)

            ones_s = const.tile([1, P], FP32)
            nc.vector.memset(ones_s[:], 1.0)
            ones_r = const.tile([1, P], FP32R)
            nc.sync.dma_start(ones_r[:], ones_in[:])
            iota_u = const.tile([P, E], U32)
            nc.gpsimd.iota(iota_u[:], pattern=[[1, E]], base=0, channel_multiplier=0)

            gws = []
            for dc in range(DC):
                g = const.tile([P, E], FP32, tag=f"gw{dc}")
                nc.sync.dma_start(g[:], gate_w[dc * P:(dc + 1) * P, :])
                gws.append(g)
            gb = const.tile([1, E], FP32)
            nc.sync.dma_start(gb[:], gate_b[:])

            cmb = _gate_combine(nc, tc, (gatep, cmbp, psg), xts, gws, gb,
                                ones_s, iota_u, TC)

            # ---- experts ----
            acc = [None] * TC
            for e in range(E):
                w2t = []
                for h in range(HC):
                    w = w2p.tile([P, D], FP32R, tag="w2t")
                    nc.sync.dma_start(w[:], w2[e, h * P:(h + 1) * P, :])
                    w2t.append(w)
                b2r = bp.tile([1, D], FP32R, tag="b2r")
                nc.sync.dma_start(b2r[:], b2[e])
                b1t = bp.tile([P, HC], FP32, tag="b1t")
                nc.sync.dma_start(b1t[:], b1p[e])

                # layer 1: hT[h] = gelu(w1[e].T-block @ x + b1)   [P, TN] per h-chunk
                hts = []
                w1e = w1[e].rearrange("(dc p) h -> p dc h", p=P)
                for h in range(HC):
                    w1t = w1p.tile([P, DC, P], FP32R, tag="w1t")
                    nc.sync.dma_start(w1t[:], w1e[:, :, h * P:(h + 1) * P])
                    p1 = ps1.tile([P, TN], FP32)
                    for dc in range(DC):
                        nc.tensor.matmul(p1[:], w1t[:, dc, :], xtr[dc][:],
                                         start=(dc == 0), stop=(dc == DC - 1))
                    ht = hp.tile([P, TN], FP32R, tag="ht")
                    nc.scalar.activation(ht[:], p1[:], AFT.Gelu_apprx_tanh,
                                         bias=b1t[:, h:h + 1])
                    hts.append(ht)

                # layer 2: y[t-chunk] = hT.T @ w2[e] + b2 ; out-accumulate scaled
                for t in range(TC):
                    p2 = ps2.tile([P, D], FP32)
                    for h in range(HC):
                        nc.tensor.matmul(p2[:], hts[h][:, t * P:(t + 1) * P], w2t[h][:],
                                         start=(h == 0), stop=False)
                    nc.tensor.matmul(p2[:], ones_r[:], b2r[:], start=False, stop=True)
                    if e == 0:
                        a = accp.tile([P, D], FP32, tag="acc")
                        nc.vector.tensor_scalar_mul(a[:], p2[:], cmb[t][:, e:e + 1])
                        acc[t] = a
                    else:
                        tmp = tmpp.tile([P, D], FP32, tag="tmp")
                        nc.scalar.activation(tmp[:], p2[:], AFT.Copy,
                                             scale=cmb[t][:, e:e + 1])
                        nc.vector.tensor_add(acc[t][:], acc[t][:], tmp[:])

            for t in range(TC):
                nc.sync.dma_start(out[t * P:(t + 1) * P, :], acc[t][:])

    nc.compile()
    return nc


_NC_CACHE = {}


def _get_nc():
    if "dense" not in _NC_CACHE:
        _NC_CACHE["dense"] = build_dense()
    return _NC_CACHE["dense"]


def make_in_maps(inp, gate_w, gate_b, w1, b1, w2, b2):
    inp = np.ascontiguousarray(np.asarray(inp, dtype=np.float32))
    gate_w = np.ascontiguousarray(np.asarray(gate_w, dtype=np.float32))
    gate_b = np.ascontiguousarray(np.asarray(gate_b, dtype=np.float32)).reshape(1, E)
    w1 = np.ascontiguousarray(np.asarray(w1, dtype=np.float32))
    b1 = np.ascontiguousarray(np.asarray(b1, dtype=np.float32))
    w2 = np.ascontiguousarray(np.asarray(w2, dtype=np.float32))
    b2 = np.ascontiguousarray(np.asarray(b2, dtype=np.float32)).reshape(E, 1, D)
    # b1p[e, p, j] = b1[e, j*128 + p]
    b1p = np.ascontiguousarray(b1.reshape(E, HC, P).transpose(0, 2, 1))

    in_maps = []
    for c in range(M):
        xT = np.ascontiguousarray(inp[c * TN:(c + 1) * TN, :].T)
        in_maps.append({
            "xT_r": xT, "xT_s": xT,
            "gate_w": gate_w, "gate_b": gate_b,
            "w1": w1, "b1p": b1p, "w2": w2, "b2": b2,
            "ones_in": np.ones((1, P), np.float32),
        })
    return in_maps


def run(inputs, trace=False, **spmd_kwargs):
    nc = _get_nc()
    in_maps = make_in_maps(
        inputs["inp"], inputs["gate_w"], inputs["gate_b"],
        inputs["w1"], inputs["b1"], inputs["w2"], inputs["b2"])
    res = run_bass_kernel_spmd(nc, in_maps, list(range(M)), trace=trace, **spmd_kwargs)
    out = np.concatenate([res.results[c]["out"] for c in range(M)], axis=0)
    return out, res


def kernel(inp, gate_w, gate_b, w1, b1, w2, b2, top_k):
    assert int(top_k) == TOPK
    out, _ = run({"inp": inp, "gate_w": gate_w, "gate_b": gate_b,
                  "w1": w1, "b1": b1, "w2": w2, "b2": b2})
    return out
